# revision 1
# baseline (speedup 1.0000x reference)
"""CrossAttention TRN2 kernel: 8 cores = (batch 4) x (head-group 2).

Layout strategy (per core, batch b, 8 heads g):
  host: transpose x,y,weights; fold softmax bias as exp(bias)^T (bf16);
  device: QKV projections in fp32r; scores computed transposed S^T[t',q]
  with K=64 row-tiled head pairs; k-norm folded into exp's per-partition
  scale; q-norm applied via selector-matmul broadcast; P*V with an ones
  column giving the softmax denominator; out-projection back to [q,c].
  host: sum the two head-group partial products per batch + p_bias.
"""
import sys
if '/opt/trn_rl_repo' not in sys.path:
    sys.path.insert(0, '/opt/trn_rl_repo')
import math
import numpy as np
import ml_dtypes

B, L, IN = 4, 2048, 1024
H, DH = 16, 64          # total heads, head dim
HG = 8                  # heads per core
EG = HG * DH            # embed per core = 512
NBLK = EG // 128        # 4 dh-blocks per core (2 heads each)
NQC = L // 512          # 4 q chunks of 512
NTC = L // 128          # 16 t' chunks of 128
NKC = IN // 128         # 8 contraction chunks
EPS = 1e-12
MAXSM = math.log(100.0)

_CACHE = {}


def _build_nc():
    import concourse.bass as bass
    import concourse.mybir as mybir
    import concourse.tile as tile

    f32 = mybir.dt.float32
    f32r = mybir.dt.float32r
    bf16 = mybir.dt.float16  # fp16: 10-bit mantissa, p<1e4 fits range
    A = mybir.AluOpType
    AF = mybir.ActivationFunctionType

    nc = bass.Bass()
    dt_in = [
        ("xT", [IN, L], f32), ("yT", [IN, L], f32),
        ("WqT", [IN, EG], f32), ("WkT", [IN, EG], f32), ("WvT", [IN, EG], f32),
        ("WpT", [EG, IN], f32), ("qb", [128, NBLK], f32),
        ("smp", [2, NBLK], f32), ("eBT", [NQC, L, 512], bf16),
        ("esel", [64, 128], f32), ("E2", [128, 2], f32), ("E2T", [2, 128], f32),
    ]
    d = {n: nc.dram_tensor(n, s, t, kind="ExternalInput") for n, s, t in dt_in}
    out_d = nc.dram_tensor("out", [L, IN], f32, kind="ExternalOutput")

    with tile.TileContext(nc) as tc:
        with tc.tile_pool(name="persist", bufs=1) as pp:
            # persistent tiles
            kTn = pp.tile([128, NBLK, L], f32r, name="kTn")
            vsb = pp.tile([128, NTC, HG * (DH + 1)], bf16, name="vsb")
            oTn = pp.tile([128, NBLK, L], f32r, name="oTn")
            aT = pp.tile([128, NBLK, NTC, 2], f32, name="aT")
            esel_s = pp.tile([64, 128], f32r, name="esel_s")
            E2_s = pp.tile([128, 2], f32r, name="E2_s")
            E2T_s = pp.tile([2, 128], f32r, name="E2T_s")
            smp_s = pp.tile([2, NBLK], f32, name="smp_s")
            qb_s = pp.tile([128, NBLK], f32, name="qb_s")
            nc.gpsimd.dma_start(esel_s[:], d["esel"][:])
            nc.gpsimd.dma_start(E2_s[:], d["E2"][:])
            nc.gpsimd.dma_start(E2T_s[:], d["E2T"][:])
            nc.sync.dma_start(smp_s[:], d["smp"][:])
            nc.sync.dma_start(qb_s[:], d["qb"][:])
            nc.vector.memset(vsb.rearrange("p t (h e) -> p t h e", e=DH + 1)[:, :, :, DH], 1.0)

            # ---- phase 1: k^T, v, norms from yT ----
            with tc.tile_pool(name="ph1", bufs=1) as ph1, \
                 tc.tile_pool(name="ph1w", bufs=1) as ph1w, \
                 tc.tile_pool(name="psproj", bufs=3, space="PSUM") as psproj, \
                 tc.tile_pool(name="psn", bufs=2, space="PSUM") as psn:
                yTs = ph1.tile([128, NKC, L], f32r, name="yTs")
                nc.gpsimd.dma_start(yTs[:], d["yT"].rearrange("(o p) t -> p o t", p=128))
                WkTs = ph1.tile([128, NKC, EG], f32r, name="WkTs")
                nc.gpsimd.dma_start(WkTs[:], d["WkT"].rearrange("(o p) e -> p o e", p=128))
                WvTs = ph1.tile([128, NKC, EG], f32r, name="WvTs")
                nc.gpsimd.dma_start(WvTs[:], d["WvT"].rearrange("(o p) e -> p o e", p=128))
                # k^T blocks: [dh_128, t_512] = sum_k WkT[k,dh128].T @ yT[k,t512]
                for blk in range(NBLK):
                    for t5 in range(NQC):
                        ps = psproj.tile([128, 512], f32, name="kps", tag="proj")
                        for k in range(NKC):
                            nc.tensor.matmul(
                                ps[:], WkTs[:, k, blk * 128:(blk + 1) * 128],
                                yTs[:, k, t5 * 512:(t5 + 1) * 512],
                                start=(k == 0), stop=(k == NKC - 1))
                        nc.scalar.copy(kTn[:, blk, t5 * 512:(t5 + 1) * 512], ps[:])
                    # squares + per-head colsums -> ssqT in psum [t'128, 2] x16
                    ksq = ph1w.tile([128, L], f32r, name="ksq", tag="ksq")
                    nc.vector.tensor_tensor(ksq[:], kTn[:, blk], kTn[:, blk], A.mult)
                    sq = psn.tile([128, 2 * NTC], f32, name="sqk", tag="sq")
                    for c in range(NTC):
                        nc.tensor.matmul(
                            sq[:, 2 * c:2 * c + 2], ksq[:, c * 128:(c + 1) * 128],
                            E2_s[:], start=True, stop=True)
                    nrm = ph1w.tile([128, 2 * NTC], f32, name="nrmk", tag="nrmk")
                    nc.scalar.activation(nrm[:], sq[:], AF.Sqrt)
                    nc.vector.tensor_scalar(nrm[:], nrm[:], EPS, None, A.max)
                    nc.vector.reciprocal(
                        aT[:, blk].rearrange("p a b -> p (a b)"), nrm[:])
                # v: [t'_128, dh_512] = sum_k yT[k, t128].T @ WvT[k, :]
                vr = vsb.rearrange("p t (h e) -> p t h e", e=DH + 1)
                for tb in range(NTC):
                    ps = psproj.tile([128, 512], f32, name="vps", tag="proj")
                    for k in range(NKC):
                        nc.tensor.matmul(
                            ps[:], yTs[:, k, tb * 128:(tb + 1) * 128],
                            WvTs[:, k], start=(k == 0), stop=(k == NKC - 1))
                    for h2 in range(2):
                        nc.scalar.copy(
                            vr[:, tb, h2 * 4:(h2 + 1) * 4, 0:DH],
                            ps[:, h2 * 256:(h2 + 1) * 256].rearrange(
                                "p (h e) -> p h e", e=DH))

            with tc.tile_pool(name="pp23", bufs=1) as pp23:
                # ---- phase 2: q^T + q-norm from xT (streamed per q-chunk) ----
                qTn = pp23.tile([128, NBLK, L], f32r, name="qTn")
                WpTs = pp23.tile([128, NBLK, IN], f32r, name="WpTs")
                nc.gpsimd.dma_start(WpTs[:], d["WpT"].rearrange("(o p) c -> p o c", p=128))
                with tc.tile_pool(name="ph2", bufs=1) as ph2, \
                     tc.tile_pool(name="ph2w", bufs=2) as ph2w, \
                     tc.tile_pool(name="psproj", bufs=3, space="PSUM") as psproj, \
                     tc.tile_pool(name="psn2", bufs=2, space="PSUM") as psn2:
                    WqTs = ph2.tile([128, NKC, EG], f32r, name="WqTs")
                    nc.gpsimd.dma_start(WqTs[:], d["WqT"].rearrange("(o p) e -> p o e", p=128))
                    xTr = d["xT"].rearrange("(o p) t -> p o t", p=128)
                    for t5 in range(NQC):
                        xsl = ph2w.tile([128, NKC, 512], f32r, name="xsl", tag="xsl")
                        nc.gpsimd.dma_start(xsl[:], xTr[:, :, t5 * 512:(t5 + 1) * 512])
                        for blk in range(NBLK):
                            ps = psproj.tile([128, 512], f32, name="qps", tag="proj")
                            for k in range(NKC):
                                nc.tensor.matmul(
                                    ps[:], WqTs[:, k, blk * 128:(blk + 1) * 128],
                                    xsl[:, k], start=(k == 0), stop=(k == NKC - 1))
                            qTc = ph2w.tile([128, 512], f32, name="qTc", tag="qTc")
                            nc.scalar.activation(
                                qTc[:], ps[:], AF.Identity,
                                bias=qb_s[:, blk:blk + 1], scale=1.0)
                            qsq = ph2w.tile([128, 512], f32r, name="qsq", tag="qsq")
                            nc.vector.tensor_tensor(qsq[:], qTc[:], qTc[:], A.mult)
                            sq = psn2.tile([2, 512], f32, name="sqq", tag="sqq")
                            nc.tensor.matmul(sq[:], E2_s[:], qsq[:], start=True, stop=True)
                            bi = ph2w.tile([2, 512], f32, name="bi", tag="bi")
                            nc.scalar.activation(bi[:], sq[:], AF.Sqrt)
                            nc.vector.tensor_scalar(bi[:], bi[:], EPS, None, A.max)
                            bir = ph2w.tile([2, 512], f32r, name="bir", tag="bir")
                            with nc.allow_low_precision(reason="q norm scale"):
                                nc.vector.reciprocal(bir[:], bi[:])
                            nc.scalar.activation(
                                bir[:], bir[:], AF.Copy, bias=0.0,
                                scale=smp_s[:, blk:blk + 1])
                            bb = psn2.tile([128, 512], f32, name="bb", tag="bb")
                            nc.tensor.matmul(bb[:], E2T_s[:], bir[:], start=True, stop=True)
                            nc.vector.tensor_tensor(
                                qTn[:, blk, t5 * 512:(t5 + 1) * 512],
                                qTc[:], bb[:], A.mult)

                # ---- phase 3: attention + out-projection ----
                with tc.tile_pool(name="mn", bufs=1) as mn, \
                     tc.tile_pool(name="pTp", bufs=16) as pTp, \
                     tc.tile_pool(name="eBp", bufs=1) as eBp, \
                     tc.tile_pool(name="lp", bufs=2) as lp, \
                     tc.tile_pool(name="osb", bufs=3) as osb, \
                     tc.tile_pool(name="pss", bufs=3, space="PSUM") as pss, \
                     tc.tile_pool(name="psv", bufs=1, space="PSUM") as psv, \
                     tc.tile_pool(name="psl", bufs=1, space="PSUM") as psl, \
                     tc.tile_pool(name="pso", bufs=2, space="PSUM") as pso:
                    vr = vsb.rearrange("p t (h e) -> p t h e", e=DH + 1)
                    for qc in range(NQC):
                        eB = eBp.tile([128, NTC, 512], bf16, name="eB", tag="eB")
                        nc.sync.dma_start(
                            eB[:], d["eBT"][qc].rearrange("(c p) q -> p c q", p=128))
                        for pair in range(NBLK):
                            oT = [psv.tile([DH + 1, 512], f32, name=f"oT{h}", tag=f"oT{h}")
                                  for h in range(2)]
                            pTs = []
                            for c in range(NTC):
                                row = []
                                for h in range(2):
                                    ss = pss.tile([128, 512], f32, name="ss", tag="ss")
                                    nc.tensor.matmul(
                                        ss[:],
                                        kTn[h * 64:(h + 1) * 64, pair, c * 128:(c + 1) * 128],
                                        qTn[h * 64:(h + 1) * 64, pair, qc * 512:(qc + 1) * 512],
                                        start=True, stop=True)
                                    pT = pTp.tile([128, 512], bf16, name=f"pT{h}", tag=f"pT{h}")
                                    nc.scalar.activation(
                                        pT[:], ss[:], AF.Exp, bias=0.0,
                                        scale=aT[:, pair, c:c + 1, h])
                                    nc.vector.tensor_tensor(pT[:], pT[:], eB[:, c], A.mult)
                                    row.append(pT)
                                pTs.append(row)
                            for c in range(NTC):
                                for h in range(2):
                                    nc.tensor.matmul(
                                        oT[h][:], vr[:, c, pair * 2 + h, :],
                                        pTs[c][h][:],
                                        start=(c == 0), stop=(c == NTC - 1))
                            linv = lp.tile([64, 512], f32r, name="linv", tag="linv")
                            with nc.allow_low_precision(reason="1/l bcast"):
                                for h in range(2):
                                    nc.vector.reciprocal(
                                        linv[h * 32:h * 32 + 1, :], oT[h][DH:DH + 1, :])
                            lb = psl.tile([128, 512], f32, name="lb", tag="lb")
                            nc.tensor.matmul(lb[:], esel_s[:], linv[:], start=True, stop=True)
                            lbs = lp.tile([128, 512], f32, name="lbs", tag="lbs")
                            nc.vector.tensor_copy(lbs[:], lb[:])
                            for h in range(2):
                                nc.vector.tensor_tensor(
                                    oTn[h * 64:(h + 1) * 64, pair, qc * 512:(qc + 1) * 512],
                                    oT[h][0:DH, :], lbs[h * 64:(h + 1) * 64, :], A.mult)
                        # out-projection for this q chunk
                        for q1 in range(4):
                            qoff = qc * 512 + q1 * 128
                            for cs in range(2):
                                ps = pso.tile([128, 512], f32, name="ops", tag="ops")
                                for pair in range(NBLK):
                                    nc.tensor.matmul(
                                        ps[:], oTn[:, pair, qoff:qoff + 128],
                                        WpTs[:, pair, cs * 512:(cs + 1) * 512],
                                        start=(pair == 0), stop=(pair == NBLK - 1))
                                ob = osb.tile([128, 512], f32, name="ob", tag="ob")
                                nc.scalar.copy(ob[:], ps[:])
                                nc.sync.dma_start(
                                    out_d[qoff:qoff + 128, cs * 512:(cs + 1) * 512], ob[:])
    _split_excess_waits(nc)
    return nc


def _split_excess_waits(nc):
    import concourse.mybir as mybir
    for f in nc.m.functions:
        for bb in f.blocks:
            new_insts = []
            for inst in bb.instructions:
                si = inst.sync_info
                if si is not None and si.on_wait and len(si.on_wait) > 1:
                    waits = list(si.on_wait)
                    for ci, w in enumerate(waits[:-1]):
                        new_insts.append(mybir.InstNoOp(
                            name=f"{inst.name}-ws{ci}", engine=inst.engine,
                            ins=[], outs=[],
                            sync_info=mybir.SyncInfo(on_wait=[w], on_update=[])))
                    inst.sync_info = mybir.SyncInfo(
                        on_wait=waits[-1:], on_update=si.on_update)
                new_insts.append(inst)
            bb.instructions[:] = new_insts


def kernel(x, y, attn_bias, Wq, Wk, Wv, q_bias, scale_mul, Wp, p_bias):
    from concourse.bass_utils import run_bass_kernel_spmd
    if "nc" not in _CACHE:
        _CACHE["nc"] = _build_nc()
    nc = _CACHE["nc"]

    x = np.asarray(x, dtype=np.float32)
    y = np.asarray(y, dtype=np.float32)
    bias = np.asarray(attn_bias, dtype=np.float32)[0, 0]
    Wq = np.asarray(Wq, dtype=np.float32); Wk = np.asarray(Wk, dtype=np.float32)
    Wv = np.asarray(Wv, dtype=np.float32); Wp = np.asarray(Wp, dtype=np.float32)
    q_bias = np.asarray(q_bias, dtype=np.float32)
    p_bias = np.asarray(p_bias, dtype=np.float32)
    sm = np.exp(np.minimum(np.asarray(scale_mul, dtype=np.float32), MAXSM))[0, :, 0, 0]

    # shared host prep
    eBT_f = np.exp(bias.T)                            # [t', q]
    eBT = np.ascontiguousarray(
        eBT_f.reshape(L, NQC, 512).transpose(1, 0, 2)).astype(np.float16)
    esel = np.zeros((64, 128), np.float32); esel[0, 0:64] = 1; esel[32, 64:128] = 1
    E2 = np.zeros((128, 2), np.float32); E2[0:64, 0] = 1; E2[64:128, 1] = 1
    E2T = np.ascontiguousarray(E2.T)
    xT = [np.ascontiguousarray(x[b].T) for b in range(B)]
    yT = [np.ascontiguousarray(y[b].T) for b in range(B)]

    in_maps = []
    for core in range(8):
        b, g = core // 2, core % 2
        sl = slice(g * EG, (g + 1) * EG)
        smp = np.ascontiguousarray(
            sm[g * HG:(g + 1) * HG].reshape(NBLK, 2).T)    # [2, NBLK]
        qb_t = np.ascontiguousarray(
            q_bias[sl].reshape(NBLK, 128).T)               # [128, NBLK]
        in_maps.append({
            "xT": xT[b], "yT": yT[b],
            "WqT": np.ascontiguousarray(Wq[sl].T),
            "WkT": np.ascontiguousarray(Wk[sl].T),
            "WvT": np.ascontiguousarray(Wv[sl].T),
            "WpT": np.ascontiguousarray(Wp[:, sl].T),
            "qb": qb_t, "smp": smp, "eBT": eBT,
            "esel": esel, "E2": E2, "E2T": E2T,
        })
    res = run_bass_kernel_spmd(nc, in_maps, core_ids=list(range(8)))
    parts = [r["out"] for r in res.results]
    out = np.empty((B, L, IN), np.float32)
    for b in range(B):
        out[b] = parts[2 * b] + parts[2 * b + 1] + p_bias
    return out

